# revision 8
# baseline (speedup 1.0000x reference)
"""NT-Xent loss kernel for 8 Trainium2 NeuronCores (Bass/Tile).

Strategy (data-parallel rows, SPMD):
  - Host: concat z_i,z_j -> reps [8192, 512], cast bf16. Core c receives
    np.roll(reps, -c*1024, axis=0) so every core runs the same static
    program on "its" first 1024 rows: self-similarity for local row li
    sits at column li, the positive partner at column li+4096.
  - On-chip per core: row squared-norms via fused DVE multiply+accumulate,
    inv-norm via Scalar ln/exp (one activation table set, no reloads),
    normalize rows on DVE, transpose via XBAR DMA-transpose into a k-major
    repsT [128, KT, N] (bf16), then the [1024, 8192] similarity block as
    [128, 2048] PSUM tiles (bf16 matmul, f32 accum, 1024-wide moving
    operand). Self column masked with a -1e30 eye tile; exp(4*sim-4) on
    ScalarE with fused row-sum accumulation; row-max via fused DVE
    tensor_tensor_reduce over exp-tile pairs (chained through the reduce
    init scalar).
  - Host: combine per-core stats (positives, hardest negatives, exp sums)
    in float64 into the scalar loss (the two "all-reduced" loss terms).
"""

import numpy as np
import ml_dtypes

import concourse.bacc as bacc
import concourse.bass as bass
import concourse.tile as tile
import concourse.mybir as mybir
from concourse.bass_utils import run_bass_kernel_spmd

B = 4096
D = 512
N = 2 * B            # 8192 rows total
NCORES = 8
NLOC = N // NCORES   # 1024 rows per core
RT = N // 128        # 64 row tiles
MT = NLOC // 128     # 8 local row tiles
KT = D // 128        # 4 contraction chunks
NG = 8               # row-tile groups (8 r-tiles each)
SW = 2048            # column super-tile width (4 PSUM banks)
NS = N // SW         # 4 column super-tiles

F32 = mybir.dt.float32
BF16 = mybir.dt.bfloat16

_CACHE = {}


def _build_program():
    if "nc" in _CACHE:
        return _CACHE["nc"]
    nc = bacc.Bacc(
        "TRN2",
        target_bir_lowering=False,
        debug=False,
        num_devices=NCORES,
    )

    z = nc.dram_tensor("z", [N, D], BF16, kind="ExternalInput").ap()
    negeye = nc.dram_tensor("negeye", [128, 128], F32, kind="ExternalInput").ap()

    mx_d = nc.dram_tensor("mx", [128, MT], F32, kind="ExternalOutput").ap()
    esum_d = nc.dram_tensor("esum", [128, MT, NS], F32, kind="ExternalOutput").ap()
    posd_d = nc.dram_tensor("posd", [128, MT], F32, kind="ExternalOutput").ap()
    ssq_d = nc.dram_tensor("ssq", [128, RT], F32, kind="ExternalOutput").ap()

    ALU = mybir.AluOpType
    AF = mybir.ActivationFunctionType

    with tile.TileContext(nc) as tc:
        with (
            tc.tile_pool(name="persist", bufs=1) as persist,
            tc.tile_pool(name="nrows", bufs=2) as nrows,
            tc.tile_pool(name="sqtr", bufs=2) as sqtrp,
            tc.tile_pool(name="etodd", bufs=3) as etoddp,
            tc.tile_pool(name="mm", bufs=2, space="PSUM") as mmp,
        ):
            zfull = persist.tile([128, RT, 512], BF16, tag="zfull")
            # k-major transposed reps: repsT[p, k, col] = feature k*128+p of
            # local row col  (col = rblk*256 + sub*128 + c from the XBAR)
            repsT = persist.tile([128, KT, N], BF16, tag="repsT")
            negeyeS = persist.tile([128, 128], F32, tag="negeyeS")
            ssqall = persist.tile([128, RT], F32, tag="ssqall")
            rsq = persist.tile([128, RT], F32, tag="rsq")
            lnssq = persist.tile([128, RT], F32, tag="lnssq")
            invall = persist.tile([128, RT], F32, tag="invall")
            posdt = persist.tile([128, MT], F32, tag="posdt")
            mxf = persist.tile([128, MT], F32, tag="mxf")
            esm = persist.tile([128, MT, NS], F32, tag="esm")
            # even column-supertile exp tiles stay resident for the TTR pair
            etev = persist.tile([128, MT, SW], BF16, tag="etev")
            negfour = persist.tile([128, 1], F32, tag="negfour")

            nc.vector.memset(negfour, -4.0)
            nc.sync.dma_start(out=negeyeS, in_=negeye)

            def prep_dma(g):
                nc.sync.dma_start(
                    out=zfull[:, g * 8 : (g + 1) * 8, :],
                    in_=z[g * 1024 : (g + 1) * 1024, :].rearrange(
                        "(j p) f -> p j f", p=128
                    ),
                )

            def prep(g):
                """ssq + inv-norm + normalized rows + XBAR transpose, group g."""
                gs = slice(g * 8, g * 8 + 8)
                for r in range(g * 8, g * 8 + 8):
                    sq = sqtrp.tile([128, 512], BF16, tag="sqtr")
                    nc.vector.scalar_tensor_tensor(
                        out=sq,
                        in0=zfull[:, r, :],
                        scalar=1.0,
                        in1=zfull[:, r, :],
                        op0=ALU.mult,
                        op1=ALU.mult,
                        accum_out=ssqall[:, r : r + 1],
                    )
                # inv = exp(-0.5*ln(ssq)); ln+exp live in one act table set
                nc.scalar.activation(lnssq[:, gs], ssqall[:, gs], AF.Ln)
                nc.scalar.activation(invall[:, gs], lnssq[:, gs], AF.Exp, scale=-0.5)
                nrow = nrows.tile([128, 8, 512], BF16, tag="nrow")
                for j in range(8):
                    r = g * 8 + j
                    nc.vector.tensor_scalar_mul(
                        nrow[:, j, :], zfull[:, r, :], invall[:, r : r + 1]
                    )
                    # XBAR transpose into k-major repsT (out free dims (k, c))
                    nc.sync.dma_start(
                        out=repsT[:, :, r * 128 : (r + 1) * 128],
                        in_=nrow[:, j, :],
                        transpose=True,
                    )

            def positives():
                for q in range(MT):
                    sq = sqtrp.tile([128, 512], BF16, tag="sqtr")
                    nc.vector.scalar_tensor_tensor(
                        out=sq,
                        in0=zfull[:, q, :],
                        scalar=1.0,
                        in1=zfull[:, 32 + q, :],
                        op0=ALU.mult,
                        op1=ALU.mult,
                        accum_out=posdt[:, q : q + 1],
                    )

            def main_m(G, m):
                ps = mmp.tile([128, SW], F32, tag="ps")
                for k in range(KT):
                    for h in range(4):
                        nc.tensor.matmul(
                            ps[:, h * 512 : (h + 1) * 512],
                            lhsT=repsT[:, k, m * 128 : (m + 1) * 128],
                            rhs=repsT[
                                :, k, G * SW + h * 512 : G * SW + (h + 1) * 512
                            ],
                            start=(k == 0),
                            stop=(k == KT - 1),
                        )
                if G == 0:
                    # mask self-similarity: sim[p, m*128+p] -= 1e30
                    nc.vector.tensor_add(
                        ps[:, m * 128 : (m + 1) * 128],
                        ps[:, m * 128 : (m + 1) * 128],
                        negeyeS,
                    )
                if G == 0:
                    et = etev[:, m, :]
                else:
                    et = etoddp.tile([128, SW], BF16, tag="etodd")
                nc.scalar.activation(
                    out=et,
                    in_=ps,
                    func=AF.Exp,
                    bias=negfour,
                    scale=4.0,
                    accum_out=esm[:, m, G : G + 1],
                )
                if G > 0:
                    # running elementwise max into the resident G=0 tile (2x bf16)
                    nc.vector.tensor_max(etev[:, m, :], etev[:, m, :], et)
                if G == NS - 1:
                    nc.vector.reduce_max(
                        mxf[:, m : m + 1], etev[:, m, :], axis=mybir.AxisListType.X
                    )

            # ---- schedule ----
            for g in range(NG):
                prep_dma(g)
            prep(0)
            prep(1)
            for G in range(NS):
                for m in range(MT):
                    main_m(G, m)
                    if G < NS - 1 and m == 3:
                        prep(2 * G + 2)
                    if G < NS - 1 and m == 5:
                        prep(2 * G + 3)
                    if G == 2 and m == 6:
                        positives()

            nc.sync.dma_start(out=mx_d, in_=mxf)
            nc.sync.dma_start(out=esum_d, in_=esm)
            nc.sync.dma_start(out=posd_d, in_=posdt)
            nc.sync.dma_start(out=ssq_d, in_=ssqall)

    nc.compile()
    _CACHE["nc"] = nc
    return nc


def _host_inputs(z_i, z_j):
    reps = np.concatenate(
        [np.asarray(z_i, np.float32), np.asarray(z_j, np.float32)], axis=0
    )
    zb = reps.astype(ml_dtypes.bfloat16)
    negeye = (np.eye(128, dtype=np.float32) * -1.0e30).astype(np.float32)
    in_maps = []
    for c in range(NCORES):
        zc = np.ascontiguousarray(np.roll(zb, -c * NLOC, axis=0))
        in_maps.append({"z": zc, "negeye": negeye})
    return in_maps


def _combine(results):
    pos = np.zeros(N, np.float64)
    hn = np.zeros(N, np.float64)
    S = 0.0
    for c, o in enumerate(results):
        mx = np.asarray(o["mx"], np.float64)       # [128, MT]
        esum = np.asarray(o["esum"], np.float64)   # [128, MT, NS]
        posd = np.asarray(o["posd"], np.float64)   # [128, MT]
        ssq = np.asarray(o["ssq"], np.float64)     # [128, RT]
        # mx holds max over exp(4*sim-4) (bf16 rounded); invert the exp.
        hn_loc = (np.log(mx.T.reshape(NLOC)) + 4.0) / 4.0
        S += esum.sum()                            # self terms exp'd to 0
        invrow = 1.0 / np.sqrt(ssq.T.reshape(N))   # rolled row index
        posl = posd.T.reshape(NLOC) * invrow[:NLOC] * invrow[B : B + NLOC]
        gl = (np.arange(NLOC) + c * NLOC) % N
        pos[gl] = posl
        hn[gl] = hn_loc
    ce = np.mean(np.logaddexp(0.0, 40.0 * hn - 20.0 * pos))
    npairs = N * (N - 1) // 2
    uniformity = np.log(S / 2.0 / npairs)
    return np.array(ce + 0.2 * uniformity, dtype=np.float32)


def run(z_i, z_j, **spmd_kwargs):
    nc = _build_program()
    in_maps = _host_inputs(z_i, z_j)
    res = run_bass_kernel_spmd(nc, in_maps, core_ids=list(range(NCORES)), **spmd_kwargs)
    return _combine(res.results), res


def kernel(z_i, z_j):
    loss, _ = run(z_i, z_j)
    return loss


# revision 9
# speedup vs baseline: 1.2472x; 1.2472x over previous
"""NT-Xent loss kernel for 8 Trainium2 NeuronCores (Bass/Tile).

Strategy (data-parallel rows, SPMD):
  - Host: concat z_i,z_j -> reps [8192, 512], cast bf16. Core c receives
    np.roll(reps, -c*1024, axis=0) so every core runs the same static
    program on "its" first 1024 rows: self-similarity for local row li
    sits at column li, the positive partner at column li+4096.
  - On-chip per core: row squared-norms via fused DVE multiply+accumulate
    (scalar_tensor_tensor), inv-norm via Scalar ln/exp (one activation
    table set, no table reloads), normalize rows on DVE, transpose into
    rblk-major repsT [p, rblk, sub, k, c]: groups 0-1 via PE transposes
    (fast pipeline start), groups 2-7 via one batched XBAR DMA-transpose
    each (runs on the otherwise-idle Sync engine). Similarity block
    computed as [128, 1024] PSUM tiles (bf16 matmul, f32 accum, 3-dim
    moving AP). Self column masked with a -1e30 eye tile; exp(4*sim-4)
    on ScalarE with fused row-sum accumulation; row-max via running
    elementwise tensor_max (2x bf16) + one final reduce per m-tile.
  - Host: combine per-core stats (positives, hardest negatives, exp sums)
    in float64 into the scalar loss (the two "all-reduced" loss terms).
"""

import numpy as np
import ml_dtypes

import concourse.bacc as bacc
import concourse.bass as bass
import concourse.tile as tile
import concourse.mybir as mybir
from concourse.bass_utils import run_bass_kernel_spmd

B = 4096
D = 512
N = 2 * B            # 8192 rows total
NCORES = 8
NLOC = N // NCORES   # 1024 rows per core
RT = N // 128        # 64 row tiles
MT = NLOC // 128     # 8 local row tiles
KT = D // 128        # 4 contraction chunks
NG = 8               # row-tile groups (8 r-tiles each) == column supertiles

F32 = mybir.dt.float32
BF16 = mybir.dt.bfloat16

_CACHE = {}


def _build_program():
    if "nc" in _CACHE:
        return _CACHE["nc"]
    nc = bacc.Bacc(
        "TRN2",
        target_bir_lowering=False,
        debug=False,
        num_devices=NCORES,
    )

    z = nc.dram_tensor("z", [N, D], BF16, kind="ExternalInput").ap()
    ident = nc.dram_tensor("ident", [128, 128], BF16, kind="ExternalInput").ap()
    negeye = nc.dram_tensor("negeye", [128, 128], F32, kind="ExternalInput").ap()

    mx_d = nc.dram_tensor("mx", [128, MT], F32, kind="ExternalOutput").ap()
    esum_d = nc.dram_tensor("esum", [128, MT, NG], F32, kind="ExternalOutput").ap()
    posd_d = nc.dram_tensor("posd", [128, MT], F32, kind="ExternalOutput").ap()
    ssq_d = nc.dram_tensor("ssq", [128, RT], F32, kind="ExternalOutput").ap()

    ALU = mybir.AluOpType
    AF = mybir.ActivationFunctionType
    AX = mybir.AxisListType

    with tile.TileContext(nc) as tc:
        with (
            tc.tile_pool(name="persist", bufs=1) as persist,
            tc.tile_pool(name="nrows", bufs=2) as nrows,
            tc.tile_pool(name="sqtr", bufs=2) as sqtrp,
            tc.tile_pool(name="etodd", bufs=3) as etoddp,
            tc.tile_pool(name="pstr", bufs=2, space="PSUM") as pstrp,
            tc.tile_pool(name="mm", bufs=3, space="PSUM") as mmp,
        ):
            zfull = persist.tile([128, RT, 512], BF16, tag="zfull")
            # rblk-major transposed reps:
            # repsT[p, rblk, sub, k, c] = feature k*128+p of local row
            #   (rblk*2+sub)*128 + c
            repsT = persist.tile([128, RT // 2, 2, KT, 128], BF16, tag="repsT")
            identS = persist.tile([128, 128], BF16, tag="identS")
            negeyeS = persist.tile([128, 128], F32, tag="negeyeS")
            ssqall = persist.tile([128, RT], F32, tag="ssqall")
            lnssq = persist.tile([128, RT], F32, tag="lnssq")
            invall = persist.tile([128, RT], F32, tag="invall")
            posdt = persist.tile([128, MT], F32, tag="posdt")
            mxf = persist.tile([128, MT], F32, tag="mxf")
            esm = persist.tile([128, MT, NG], F32, tag="esm")
            # G=0 exp tiles stay resident as the running max accumulator
            etev = persist.tile([128, MT, 1024], BF16, tag="etev")
            negfour = persist.tile([128, 1], F32, tag="negfour")

            nc.vector.memset(negfour, -4.0)
            nc.sync.dma_start(out=identS, in_=ident)
            nc.sync.dma_start(out=negeyeS, in_=negeye)

            def prep_dma(g):
                nc.sync.dma_start(
                    out=zfull[:, g * 8 : (g + 1) * 8, :],
                    in_=z[g * 1024 : (g + 1) * 1024, :].rearrange(
                        "(j p) f -> p j f", p=128
                    ),
                )

            def prep_head(g):
                """ssq + inv-norm + normalized rows for group g."""
                gs = slice(g * 8, g * 8 + 8)
                for r in range(g * 8, g * 8 + 8):
                    sq = sqtrp.tile([128, 512], BF16, tag="sqtr")
                    nc.vector.scalar_tensor_tensor(
                        out=sq,
                        in0=zfull[:, r, :],
                        scalar=1.0,
                        in1=zfull[:, r, :],
                        op0=ALU.mult,
                        op1=ALU.mult,
                        accum_out=ssqall[:, r : r + 1],
                    )
                # inv = exp(-0.5*ln(ssq)); ln+exp live in one act table set
                nc.scalar.activation(lnssq[:, gs], ssqall[:, gs], AF.Ln)
                nc.scalar.activation(invall[:, gs], lnssq[:, gs], AF.Exp, scale=-0.5)
                nrow = nrows.tile([128, 8, 512], BF16, tag="nrow")
                for j in range(8):
                    r = g * 8 + j
                    nc.vector.tensor_scalar_mul(
                        nrow[:, j, :], zfull[:, r, :], invall[:, r : r + 1]
                    )
                return nrow

            def prep_pe(g):
                """groups for the pipeline head: PE transpose + DVE copy."""
                nrow = prep_head(g)
                for j in range(8):
                    r = g * 8 + j
                    pstr = pstrp.tile([128, KT, 128], BF16, tag="pstr")
                    for k in range(KT):
                        nc.tensor.transpose(
                            pstr[:, k, :], nrow[:, j, k * 128 : (k + 1) * 128], identS
                        )
                    nc.vector.tensor_copy(
                        out=repsT[:, r // 2, r % 2, :, :], in_=pstr
                    )

            def prep_xbar(g):
                """steady-state groups: one batched XBAR DMA-transpose."""
                nrow = prep_head(g)
                nc.sync.dma_start(
                    out=repsT[:, g * 4 : (g + 1) * 4, :, :, :],
                    in_=nrow,
                    transpose=True,
                )

            def positives():
                for q in range(MT):
                    sq = sqtrp.tile([128, 512], BF16, tag="sqtr")
                    nc.vector.scalar_tensor_tensor(
                        out=sq,
                        in0=zfull[:, q, :],
                        scalar=1.0,
                        in1=zfull[:, 32 + q, :],
                        op0=ALU.mult,
                        op1=ALU.mult,
                        accum_out=posdt[:, q : q + 1],
                    )

            def main_m(G, m):
                ps = mmp.tile([128, 1024], F32, tag="ps")
                for k in range(KT):
                    for h in (0, 1):
                        nc.tensor.matmul(
                            ps[:, h * 512 : (h + 1) * 512],
                            lhsT=repsT[:, m // 2, m % 2, k, :],
                            rhs=repsT[:, 4 * G + 2 * h : 4 * G + 2 * h + 2, :, k, :],
                            start=(k == 0),
                            stop=(k == KT - 1),
                        )
                if G == 0:
                    # mask self-similarity: sim[p, m*128+p] -= 1e30
                    nc.vector.tensor_add(
                        ps[:, m * 128 : (m + 1) * 128],
                        ps[:, m * 128 : (m + 1) * 128],
                        negeyeS,
                    )
                if G == 0:
                    et = etev[:, m, :]
                else:
                    et = etoddp.tile([128, 1024], BF16, tag="etodd")
                nc.scalar.activation(
                    out=et,
                    in_=ps,
                    func=AF.Exp,
                    bias=negfour,
                    scale=4.0,
                    accum_out=esm[:, m, G : G + 1],
                )
                if G > 0:
                    # running elementwise max into the resident G=0 tile
                    nc.vector.tensor_max(etev[:, m, :], etev[:, m, :], et)
                if G == NG - 1:
                    nc.vector.reduce_max(mxf[:, m : m + 1], etev[:, m, :], axis=AX.X)

            # ---- schedule ----
            prep_dma(0)
            prep_dma(1)
            prep_pe(0)
            prep_pe(1)
            for g in range(2, NG):
                prep_dma(g)
            for G in range(NG):
                for m in range(MT):
                    main_m(G, m)
                    if G < 6 and m == 3:
                        prep_xbar(G + 2)
                    if G == 1 and m == 6:
                        positives()

            nc.sync.dma_start(out=mx_d, in_=mxf)
            nc.sync.dma_start(out=esum_d, in_=esm)
            nc.sync.dma_start(out=posd_d, in_=posdt)
            nc.sync.dma_start(out=ssq_d, in_=ssqall)

    nc.compile()
    _CACHE["nc"] = nc
    return nc


def _host_inputs(z_i, z_j):
    reps = np.concatenate(
        [np.asarray(z_i, np.float32), np.asarray(z_j, np.float32)], axis=0
    )
    zb = reps.astype(ml_dtypes.bfloat16)
    ident = np.eye(128, dtype=np.float32).astype(ml_dtypes.bfloat16)
    negeye = (np.eye(128, dtype=np.float32) * -1.0e30).astype(np.float32)
    in_maps = []
    for c in range(NCORES):
        zc = np.ascontiguousarray(np.roll(zb, -c * NLOC, axis=0))
        in_maps.append({"z": zc, "ident": ident, "negeye": negeye})
    return in_maps


def _combine(results):
    pos = np.zeros(N, np.float64)
    hn = np.zeros(N, np.float64)
    S = 0.0
    for c, o in enumerate(results):
        mx = np.asarray(o["mx"], np.float64)       # [128, MT]
        esum = np.asarray(o["esum"], np.float64)   # [128, MT, NG]
        posd = np.asarray(o["posd"], np.float64)   # [128, MT]
        ssq = np.asarray(o["ssq"], np.float64)     # [128, RT]
        # mx holds max over exp(4*sim-4) (bf16 rounded); invert the exp.
        hn_loc = (np.log(mx.T.reshape(NLOC)) + 4.0) / 4.0
        S += esum.sum()                            # self terms exp'd to 0
        invrow = 1.0 / np.sqrt(ssq.T.reshape(N))   # rolled row index
        posl = posd.T.reshape(NLOC) * invrow[:NLOC] * invrow[B : B + NLOC]
        gl = (np.arange(NLOC) + c * NLOC) % N
        pos[gl] = posl
        hn[gl] = hn_loc
    ce = np.mean(np.logaddexp(0.0, 40.0 * hn - 20.0 * pos))
    npairs = N * (N - 1) // 2
    uniformity = np.log(S / 2.0 / npairs)
    return np.array(ce + 0.2 * uniformity, dtype=np.float32)


def run(z_i, z_j, **spmd_kwargs):
    nc = _build_program()
    in_maps = _host_inputs(z_i, z_j)
    res = run_bass_kernel_spmd(nc, in_maps, core_ids=list(range(NCORES)), **spmd_kwargs)
    return _combine(res.results), res


def kernel(z_i, z_j):
    loss, _ = run(z_i, z_j)
    return loss
